# revision 22
# baseline (speedup 1.0000x reference)
"""LightGCN encoder on 8 Trainium2 NeuronCores — v2.

Row-parallel SpMM (nodes sharded over 8 cores, edges partitioned by dest
row, per-layer AllGather of source embeddings) with a batched compute
pipeline:

- dma_gather (SWDGE) pulls per-edge source rows (f32, 256B) from DRAM
  tables as before, 1024 idx/call over 4 SWDGE queues.
- edge values are folded into the gathered data with ONE
  scalar_tensor_tensor per (superblock, range): gs = g * val  (fp16 out).
- one-hot scatter matrices are built 16 tiles at a time with ONE
  tensor_tensor is_equal against a fp16 iota ramp of 2048 (dest codes
  128*t + row are fp16-exact below 2048), replacing one DVE op per tile
  with one per 16 tiles.
- PE matmuls run in fp16 (lhsT = one-hot slice, rhs = scaled gather),
  accumulating per-dest-block f32 PSUM slices of a per-superblock tile;
  evacuation/stash/acc are one DVE op per superblock.
- AllGathers are interleaved with independent gather phases so their
  ncfw/link time hides behind compute: AG(ego h1) sits between the two
  S passes of the next layer, AG(u') after the item-range pass that
  doesn't depend on it.

Host-side preprocessing sorts/pads the edge lists per (dest-block,
source-range) — identical segment layout on every core so one SPMD
program serves all 8 — and maps source ids into the padded table
coordinates used on device."""

from contextlib import ExitStack

import numpy as np

NC = 8
EMB = 64
U = 50000
NNODES = 100000
NLAYERS = 3

ASHARD = 12500            # adj dest rows per core
ABLOCKS = 98              # ceil(12500/128)
APAD = ABLOCKS * 128      # 12544
SSHARD = 6250             # user dest rows per core
SBLOCKS = 49
SPAD = SBLOCKS * 128      # 6272
ETAB = NC * APAD          # 100352 padded ego table rows (2 half-slice tables)
UTAB = NC * SPAD          # 50176 padded user table rows
HBLK = ABLOCKS // 2       # 49 dest blocks per ego half-slice
HTAB = NC * HBLK * 128    # 50176 rows per half-slice table
RNG = 25088               # rows per gather range (int16-safe)
SBSZ = 4                  # dest blocks per superblock
WIN = 16                  # one-hot window: 16 tiles -> dest codes < 2048

A_PASSES = [[2, 3], [0, 1]]   # item ranges first (independent of u' AG)
S_PASSES = [[0], [1]]

_cache = {}


def _pad_coords_ego(col):
    """global node id -> (half-slice h, row within that slice table)."""
    c = col // ASHARD
    local = col % ASHARD
    bb = local // 128
    h = bb // HBLK
    return h, c * (HBLK * 128) + (bb % HBLK) * 128 + local % 128


def _pad_coords_u(col):
    # global user id -> row in the 8x6272-padded user table
    return (col // SSHARD) * SPAD + col % SSHARD


def _layout(seg_lens, nblocks, nranges):
    """Edge ordering: superblock -> range -> block. Returns (off[b][r], sb
    descriptors, total). All lengths are multiples of 128 and >= 128 so
    every (block, range) has at least one tile (batched PSUM evacuation
    assumes every block's accumulation chain exists)."""
    off = np.zeros((nblocks, nranges), np.int64)
    sbs = []
    pos = 0
    for sb0 in range(0, nblocks, SBSZ):
        blocks = list(range(sb0, min(sb0 + SBSZ, nblocks)))
        sb_start = pos
        rng_info = []
        for r in range(nranges):
            r_start = pos
            for b in blocks:
                off[b, r] = pos
                pos += int(seg_lens[b, r])
            rng_info.append((r_start, pos - r_start))
        sbs.append({"blocks": blocks, "start": sb_start, "end": pos,
                    "ranges": rng_info})
    return off, sbs, pos


def _group_and_fill(block, rng_id, local, within, val, nranges, off, total,
                    sbs, passes):
    """Place edges into the padded global layout. Padding slots keep
    idx=0/dest=0/val=0 (gather row 0, scaled by 0). dest codes are
    128 * (tile index within the pass window, mod WIN) + row-in-block,
    stored fp16 (exact: < 2048)."""
    idx = np.zeros(total, np.int16)
    destp = np.zeros(total, np.float16)
    v = np.zeros(total, np.float32)
    grp = block.astype(np.int64) * nranges + rng_id
    order = np.argsort(grp, kind="stable")
    g_sorted = grp[order]
    uniq, starts = np.unique(g_sorted, return_index=True)
    counts = np.diff(np.append(starts, len(order)))
    base_of = np.repeat(off.reshape(-1)[uniq], counts)
    grp_start_of = np.repeat(starts, counts)
    pos_sorted = base_of + (np.arange(len(order)) - grp_start_of)

    first_of = np.zeros(nranges, np.int64)
    for p in passes:
        for r in p:
            first_of[r] = p[0]
    rstart = np.zeros((len(sbs), nranges), np.int64)
    for i, sb in enumerate(sbs):
        for r in range(nranges):
            rstart[i, r] = sb["ranges"][r][0]
    bo = block[order] // SBSZ
    ro = rng_id[order]
    passstart = rstart[bo, first_of[ro]]
    t = ((pos_sorted - passstart) // 128) % WIN

    idx[pos_sorted] = local[order].astype(np.int16)
    destp[pos_sorted] = (t * 128 + within[order]).astype(np.float16)
    v[pos_sorted] = val[order]
    return idx, destp, v


def _wrap16(a):
    w = np.ascontiguousarray(a.reshape(-1, 16).T)  # [16, E/16]
    return np.tile(w, (8, 1))                      # replicated for 8 Q7 cores


def _wrap128(a):
    return np.ascontiguousarray(a.reshape(-1, 128).T)


def _preprocess(user_emb, item_emb, adj_rows, adj_cols, adj_vals, s_rows,
                s_cols, s_vals):
    ego0 = np.concatenate([np.asarray(user_emb), np.asarray(item_emb)], axis=0)
    ego0_pad = np.zeros((ETAB, EMB), np.float32)  # [2 half-slice tables]
    acc0s = []
    for c in range(NC):
        shard = np.zeros((APAD, EMB), np.float32)
        shard[:ASHARD] = ego0[c * ASHARD:(c + 1) * ASHARD]
        acc0s.append(shard)
        for h in range(2):
            dst = h * HTAB + c * (HBLK * 128)
            ego0_pad[dst:dst + HBLK * 128] = shard[h * HBLK * 128:
                                                   (h + 1) * HBLK * 128]

    adj_rows = np.asarray(adj_rows).astype(np.int64)
    adj_cols = np.asarray(adj_cols).astype(np.int64)
    adj_vals = np.asarray(adj_vals).astype(np.float32)
    s_rows_l = np.asarray(s_rows).astype(np.int64)
    s_cols_l = np.asarray(s_cols).astype(np.int64)
    s_vals_l = np.asarray(s_vals).astype(np.float32)

    # S' = I + S (self edges make u_new = u + S@u a pure segment-sum)
    self_dest = np.arange(U, dtype=np.int64)
    s_rows_l = np.concatenate([s_rows_l, self_dest])
    s_cols_l = np.concatenate([s_cols_l, self_dest])
    s_vals_l = np.concatenate([s_vals_l, np.ones(U, np.float32)])

    per_core = []
    for c in range(NC):
        m = (adj_rows >= c * ASHARD) & (adj_rows < (c + 1) * ASHARD)
        d = adj_rows[m] - c * ASHARD
        col = adj_cols[m]
        # adj sources: users -> u-table coords (ranges 0,1), items -> ego
        # table item half (ranges 2,3)
        is_u = col < U
        up = _pad_coords_u(np.where(is_u, col, 0))
        eh, er = _pad_coords_ego(np.where(is_u, 0, col))
        # item sources sit in rows [25088, 50176) of each half-slice table
        rng_id = np.where(is_u, up // RNG, 2 + eh)
        local = np.where(is_u, up % RNG, er - RNG)
        a = dict(block=d // 128, rng=rng_id, local=local, within=d % 128,
                 val=adj_vals[m])

        m = (s_rows_l >= c * SSHARD) & (s_rows_l < (c + 1) * SSHARD)
        d = s_rows_l[m] - c * SSHARD
        sh, sr = _pad_coords_ego(s_cols_l[m])  # user rows: [0, 25088) per half
        s = dict(block=d // 128, rng=sh, local=sr, within=d % 128,
                 val=s_vals_l[m])
        per_core.append((a, s))

    def seg_max(key, nblocks, nranges):
        lens = np.zeros((nblocks, nranges), np.int64)
        for c in range(NC):
            e = per_core[c][0 if key == "a" else 1]
            cnt = np.bincount(e["block"] * nranges + e["rng"],
                              minlength=nblocks * nranges)
            lens = np.maximum(lens, cnt.reshape(nblocks, nranges))
        return np.maximum(((lens + 127) // 128) * 128, 128)

    a_lens = seg_max("a", ABLOCKS, 4)
    s_lens = seg_max("s", SBLOCKS, 2)
    a_off, a_sbs, a_total = _layout(a_lens, ABLOCKS, 4)
    s_off, s_sbs, s_total = _layout(s_lens, SBLOCKS, 2)

    iota = np.broadcast_to(np.arange(WIN * 128, dtype=np.float16),
                           (128, WIN * 128)).copy()
    in_maps = []
    for c in range(NC):
        a, s = per_core[c]
        aidx, adest, aval = _group_and_fill(a["block"], a["rng"], a["local"],
                                            a["within"], a["val"], 4, a_off,
                                            a_total, a_sbs, A_PASSES)
        sidx, sdest, sval = _group_and_fill(s["block"], s["rng"], s["local"],
                                            s["within"], s["val"], 2, s_off,
                                            s_total, s_sbs, S_PASSES)
        in_maps.append({
            "ego0": ego0_pad,
            "acc0": acc0s[c],
            "aidx": _wrap16(aidx), "adest": _wrap128(adest),
            "aval": _wrap128(aval),
            "sidx": _wrap16(sidx), "sdest": _wrap128(sdest),
            "sval": _wrap128(sval),
            "iota": iota,
        })
    meta = dict(a_lens=a_lens, s_lens=s_lens, a_off=a_off, s_off=s_off,
                a_sbs=a_sbs, s_sbs=s_sbs, a_total=a_total, s_total=s_total)
    return in_maps, meta


def _build(meta):
    import concourse.tile as tile
    from concourse import bacc, mybir

    f32 = mybir.dt.float32
    f16 = mybir.dt.float16
    i16 = mybir.dt.int16
    nc = bacc.Bacc("TRN2", target_bir_lowering=False, debug=False,
                   num_devices=NC, num_swdge_queues=4)

    ego0_d = nc.dram_tensor("ego0", [ETAB, EMB], f32, kind="ExternalInput")
    acc0_d = nc.dram_tensor("acc0", [APAD, EMB], f32, kind="ExternalInput")
    aidx_d = nc.dram_tensor("aidx", [128, meta["a_total"] // 16], i16,
                            kind="ExternalInput")
    adest_d = nc.dram_tensor("adest", [128, meta["a_total"] // 128], f16,
                             kind="ExternalInput")
    aval_d = nc.dram_tensor("aval", [128, meta["a_total"] // 128], f32,
                            kind="ExternalInput")
    sidx_d = nc.dram_tensor("sidx", [128, meta["s_total"] // 16], i16,
                            kind="ExternalInput")
    sdest_d = nc.dram_tensor("sdest", [128, meta["s_total"] // 128], f16,
                             kind="ExternalInput")
    sval_d = nc.dram_tensor("sval", [128, meta["s_total"] // 128], f32,
                            kind="ExternalInput")
    iota_d = nc.dram_tensor("iota", [128, WIN * 128], f16,
                            kind="ExternalInput")
    out_d = nc.dram_tensor("out", [APAD, EMB], f32, kind="ExternalOutput")

    with tile.TileContext(nc) as tc, ExitStack() as ctx:
        persist = ctx.enter_context(tc.tile_pool(name="persist", bufs=1))
        metap = ctx.enter_context(tc.tile_pool(name="meta", bufs=3))
        gpool = ctx.enter_context(tc.tile_pool(name="g", bufs=3))
        gsp = ctx.enter_context(tc.tile_pool(name="gs", bufs=3))
        ohp = ctx.enter_context(tc.tile_pool(name="oh", bufs=5))
        evp = ctx.enter_context(tc.tile_pool(name="ev", bufs=4))
        psp = ctx.enter_context(tc.tile_pool(name="ps", bufs=8, space="PSUM"))
        dram = ctx.enter_context(tc.tile_pool(name="dr", bufs=1, space="DRAM"))

        iota_t = persist.tile([128, WIN, 128], f16, tag="iota")
        nc.sync.dma_start(
            out=iota_t[:],
            in_=iota_d.ap().rearrange("p (w d) -> p w d", d=128))
        acc_t = persist.tile([128, ABLOCKS, EMB], f32, tag="acc")
        nc.sync.dma_start(
            out=acc_t[:],
            in_=acc0_d.ap().rearrange("(b p) e -> p b e", p=128))
        # stashes hold one pass's partial sums until the merge pass; fp16
        # quantization (~5e-4 rel) is far inside the 2e-2 tolerance
        items_buf = persist.tile([128, ABLOCKS, EMB], f16, tag="itemsbuf",
                                 name="items_buf")
        s_stash = persist.tile([128, SBLOCKS, EMB], f16, tag="sstash",
                               name="s_stash")

        # AllGather payloads travel fp16 (halves ncfw link time); a SWDGE
        # cast-DMA expands each gathered table back to the f32 layout the
        # 256B-row dma_gather needs.
        agu_in = [dram.tile([SPAD, EMB], f16, tag=f"agui{l}", name=f"agui{l}")
                  for l in range(NLAYERS)]
        agu_out = [dram.tile([UTAB, EMB], f16, tag=f"aguo{l}", name=f"aguo{l}",
                             addr_space="Shared") for l in range(NLAYERS)]
        agu_tab = [dram.tile([UTAB, EMB], f32, tag=f"agut{l}", name=f"agut{l}")
                   for l in range(NLAYERS)]
        agego_in = [[dram.tile([HBLK * 128, EMB], f16, tag=f"agei{l}h{h}",
                               name=f"agei{l}h{h}") for h in range(2)]
                    for l in range(NLAYERS - 1)]
        agego_out = [[dram.tile([HTAB, EMB], f16, tag=f"ageo{l}h{h}",
                                name=f"ageo{l}h{h}", addr_space="Shared")
                      for h in range(2)] for l in range(NLAYERS - 1)]
        agego_tab = [[dram.tile([HTAB, EMB], f32, tag=f"aget{l}h{h}",
                                name=f"aget{l}h{h}") for h in range(2)]
                     for l in range(NLAYERS - 1)]

        self_q = [0]  # round-robin SWDGE queue counter

        def ag(in_t, out_t, tab_t):
            nc.gpsimd.collective_compute(
                "AllGather", mybir.AluOpType.bypass,
                replica_groups=[list(range(NC))],
                ins=[in_t[:].opt()], outs=[out_t[:].opt()])
            # fp16 -> f32 expansion (SWDGE cast during HBM->HBM DMA)
            nc.gpsimd.dma_start(out=tab_t[:], in_=out_t[:])

        def do_pass(sbs, lens, off, tabs, idx_d, dest_d, val_d, ranges,
                    stash_out=None, stash_in=None, out_fn=None,
                    use_acc=False):
            """One gather + scaled-one-hot + segment-sum pass over a
            contiguous `ranges` subset of the edge layout."""
            for sb in sbs:
                blocks = sb["blocks"]
                nblk = len(blocks)
                b0 = blocks[0]
                e0 = sb["ranges"][ranges[0]][0]
                e1 = sb["ranges"][ranges[-1]][0] + sb["ranges"][ranges[-1]][1]
                ncols = (e1 - e0) // 128
                idx_t = metap.tile([128, (e1 - e0) // 16], i16, tag="idx",
                                   name="idx_t")
                nc.sync.dma_start(out=idx_t[:],
                                  in_=idx_d.ap()[:, e0 // 16:e1 // 16])
                dest_t = metap.tile([128, ncols], f16, tag="dest",
                                    name="dest_t")
                nc.sync.dma_start(out=dest_t[:],
                                  in_=dest_d.ap()[:, e0 // 128:e1 // 128])
                val_t = metap.tile([128, ncols], f32, tag="val", name="val_t")
                nc.sync.dma_start(out=val_t[:],
                                  in_=val_d.ap()[:, e0 // 128:e1 // 128])
                # gathers first on the Pool queue, then the one-hot windows
                # on DVE BEFORE the gather-dependent scale op: the windows
                # only need dest_t, so DVE builds them while SWDGE gathers.
                gts = {}
                for r in ranges:
                    r0, rn = sb["ranges"][r]
                    gt = gpool.tile([128, rn // 128, EMB], f32, tag=f"g{r}",
                                    name=f"g{r}")
                    gts[r] = gt
                    # dma_gather ucode caps at 1024 indices per call; chunk
                    # and spread over the 4 SWDGE queues
                    for c0 in range(0, rn, 1024):
                        n = min(1024, rn - c0)
                        a0, a1 = r0 + c0, r0 + c0 + n
                        nc.gpsimd.dma_gather(
                            out_ap=gt[:, c0 // 128:(c0 + n) // 128, :],
                            in_ap=tabs[r],
                            idxs_ap=idx_t[:, (a0 - e0) // 16:(a1 - e0) // 16],
                            num_idxs=n, num_idxs_reg=n, elem_size=EMB,
                            queue_num=self_q[0] % 4)
                        self_q[0] += 1
                # one-hot windows for the whole (sb, pass) up front; the
                # matmuls then run block-major so each dest block's PSUM
                # accumulation chain is sequential (interleaved start/stop
                # chains within one PSUM bank corrupt each other on HW:
                # `start` clears the bank's has_written tracking).
                ohs = []
                for w0 in range(0, ncols, WIN):
                    wn = min(WIN, ncols - w0)
                    oh = ohp.tile([128, WIN, 128], f16, tag="oh", name="oh")
                    nc.vector.tensor_tensor(
                        oh[:, :wn, :],
                        dest_t[:, w0:w0 + wn].to_broadcast([128, wn, 128]),
                        iota_t[:, :wn, :],
                        mybir.AluOpType.is_equal)
                    ohs.append(oh)
                gs = {}
                for r in ranges:
                    r0, rn = sb["ranges"][r]
                    gs[r] = gsp.tile([128, rn // 128, EMB], f16, tag=f"gs{r}",
                                     name=f"gs{r}")
                    c0 = (r0 - e0) // 128
                    nc.vector.scalar_tensor_tensor(
                        gs[r][:], gts[r][:], 0.0,
                        val_t[:, c0:c0 + rn // 128].to_broadcast(
                            [128, rn // 128, EMB]),
                        mybir.AluOpType.bypass, mybir.AluOpType.mult)
                ps = psp.tile([128, nblk, EMB], f32, name="ps")
                for bi, b in enumerate(blocks):
                    bcols = []
                    for r in ranges:
                        for t in range(int(lens[b][r]) // 128):
                            c = (int(off[b][r]) + t * 128 - e0) // 128
                            gcol = (int(off[b][r]) + t * 128
                                    - sb["ranges"][r][0]) // 128
                            bcols.append((c, r, gcol))
                    for j, (c, r, gcol) in enumerate(bcols):
                        nc.tensor.matmul(
                            ps[:, bi, :],
                            lhsT=ohs[c // WIN][:, c % WIN, :],
                            rhs=gs[r][:, gcol, :],
                            start=(j == 0), stop=(j == len(bcols) - 1),
                            skip_group_check=True)
                if stash_out is not None:
                    nc.vector.tensor_copy(stash_out[:, b0:b0 + nblk, :],
                                          ps[:])
                    continue
                # ev is fp16: it feeds the fp16 AllGather payloads directly;
                # the acc add reads it mixed-dtype (quantization ~5e-4)
                ev = evp.tile([128, nblk, EMB], f16, tag="ev", name="ev")
                if stash_in is not None:
                    nc.vector.tensor_add(ev[:], ps[:],
                                         stash_in[:, b0:b0 + nblk, :])
                else:
                    nc.vector.tensor_copy(ev[:], ps[:])
                if out_fn is not None:
                    out_fn(b0, nblk, ev)
                if use_acc:
                    nc.vector.tensor_add(acc_t[:, b0:b0 + nblk, :],
                                         acc_t[:, b0:b0 + nblk, :], ev[:])

        for l in range(NLAYERS):
            # the ego "table" is two half-slice tables (blocks 0-48 / 49-97
            # of every shard); users sit in rows [0, RNG), items [RNG, 2RNG)
            if l == 0:
                halves = [ego0_d.ap()[h * HTAB:(h + 1) * HTAB]
                          for h in range(2)]
            else:
                halves = [agego_tab[l - 1][h][:] for h in range(2)]
            tabs_s = [halves[0][0:RNG], halves[1][0:RNG]]
            # S hop: u' = (I+S) @ u over the user rows of both halves.
            # Range 0 reads half 0 (its AllGather was issued at the end of
            # the previous layer); AG(half 1) is issued between the two S
            # passes so its transfer hides behind range-0 compute.
            do_pass(meta["s_sbs"], meta["s_lens"], meta["s_off"], tabs_s,
                    sidx_d, sdest_d, sval_d, ranges=[0], stash_out=s_stash)
            if l > 0:
                ag(agego_in[l - 1][1], agego_out[l - 1][1],
                   agego_tab[l - 1][1])
            def emit_agu(b0, nblk, ev, l=l):
                dst = agu_in[l][:][b0 * 128:(b0 + nblk) * 128, :]
                nc.sync.dma_start(
                    out=dst.rearrange("(b p) e -> p b e", p=128), in_=ev[:])
            do_pass(meta["s_sbs"], meta["s_lens"], meta["s_off"], tabs_s,
                    sidx_d, sdest_d, sval_d, ranges=[1], stash_in=s_stash,
                    out_fn=emit_agu)
            # adjacency hop over [u'; v]; item-source ranges (2,3) don't
            # depend on the u' AllGather, so they run first and the AG
            # transfer hides behind their compute.
            tabs = [agu_tab[l][:][0:RNG], agu_tab[l][:][RNG:2 * RNG],
                    halves[0][RNG:2 * RNG], halves[1][RNG:2 * RNG]]
            do_pass(meta["a_sbs"], meta["a_lens"], meta["a_off"], tabs,
                    aidx_d, adest_d, aval_d, ranges=[2, 3],
                    stash_out=items_buf)
            ag(agu_in[l], agu_out[l], agu_tab[l])
            last = l == NLAYERS - 1
            def emit_ego(b0, nblk, ev, l=l):
                done = 0
                while done < nblk:
                    b = b0 + done
                    h = b // HBLK
                    hb = b % HBLK
                    n = min(nblk - done, HBLK - hb)
                    dst = agego_in[l][h][:][hb * 128:(hb + n) * 128, :]
                    nc.sync.dma_start(
                        out=dst.rearrange("(b p) e -> p b e", p=128),
                        in_=ev[:, done:done + n, :])
                    done += n
            do_pass(meta["a_sbs"], meta["a_lens"], meta["a_off"], tabs,
                    aidx_d, adest_d, aval_d, ranges=[0, 1],
                    stash_in=items_buf, out_fn=None if last else emit_ego,
                    use_acc=True)
            if not last:
                # AG(half 0) now — hides behind the tail of the A-phase
                # compute; AG(half 1) goes out mid-S-phase next layer.
                ag(agego_in[l][0], agego_out[l][0], agego_tab[l][0])

        nc.vector.tensor_scalar_mul(acc_t[:], acc_t[:], 1.0 / (NLAYERS + 1))
        nc.sync.dma_start(
            out=out_d.ap().rearrange("(b p) e -> p b e", p=128), in_=acc_t[:])

    nc.compile()
    return nc


def kernel(user_emb, item_emb, adj_rows, adj_cols, adj_vals, s_rows, s_cols,
           s_vals):
    from concourse.bass_utils import run_bass_kernel_spmd

    in_maps, meta = _preprocess(user_emb, item_emb, adj_rows, adj_cols,
                                adj_vals, s_rows, s_cols, s_vals)
    key = ("v2", meta["a_total"], meta["s_total"])
    if _cache.get("key") != key:
        _cache["nc"] = _build(meta)
        _cache["key"] = key
    res = run_bass_kernel_spmd(_cache["nc"], in_maps,
                               core_ids=list(range(NC)))
    _cache["last_results"] = res
    full = np.empty((NNODES, EMB), np.float32)
    for c in range(NC):
        full[c * ASHARD:(c + 1) * ASHARD] = res.results[c]["out"][:ASHARD]
    return full[:U], full[U:]


# revision 23
# speedup vs baseline: 1.0874x; 1.0874x over previous
"""LightGCN encoder on 8 Trainium2 NeuronCores — v2.

Row-parallel SpMM (nodes sharded over 8 cores, edges partitioned by dest
row, per-layer AllGather of source embeddings) with a batched compute
pipeline:

- dma_gather (SWDGE) pulls per-edge source rows (f32, 256B) from DRAM
  tables as before, 1024 idx/call over 4 SWDGE queues.
- edge values are folded into the gathered data with ONE
  scalar_tensor_tensor per (superblock, range): gs = g * val  (fp16 out).
- one-hot scatter matrices are built 16 tiles at a time with ONE
  tensor_tensor is_equal against a fp16 iota ramp of 2048 (dest codes
  128*t + row are fp16-exact below 2048), replacing one DVE op per tile
  with one per 16 tiles.
- PE matmuls run in fp16 (lhsT = one-hot slice, rhs = scaled gather),
  accumulating per-dest-block f32 PSUM slices of a per-superblock tile;
  evacuation/stash/acc are one DVE op per superblock.
- AllGathers are interleaved with independent gather phases so their
  ncfw/link time hides behind compute: AG(ego h1) sits between the two
  S passes of the next layer, AG(u') after the item-range pass that
  doesn't depend on it.

Host-side preprocessing sorts/pads the edge lists per (dest-block,
source-range) — identical segment layout on every core so one SPMD
program serves all 8 — and maps source ids into the padded table
coordinates used on device."""

from contextlib import ExitStack

import numpy as np

NC = 8
EMB = 64
U = 50000
NNODES = 100000
NLAYERS = 3

ASHARD = 12500            # adj dest rows per core
ABLOCKS = 98              # ceil(12500/128)
APAD = ABLOCKS * 128      # 12544
SSHARD = 6250             # user dest rows per core
SBLOCKS = 49
SPAD = SBLOCKS * 128      # 6272
ETAB = NC * APAD          # 100352 padded ego table rows (2 half-slice tables)
UTAB = NC * SPAD          # 50176 padded user table rows
HBLK = ABLOCKS // 2       # 49 dest blocks per ego half-slice
HTAB = NC * HBLK * 128    # 50176 rows per half-slice table
RNG = 25088               # rows per gather range (int16-safe)
SBSZ = 4                  # dest blocks per superblock
WIN = 16                  # one-hot window: 16 tiles -> dest codes < 2048

A_PASSES = [[2, 3], [0, 1]]   # item ranges first (independent of u' AG)
S_PASSES = [[0], [1]]

_cache = {}


def _pad_coords_ego(col):
    """global node id -> (half-slice h, row within that slice table)."""
    c = col // ASHARD
    local = col % ASHARD
    bb = local // 128
    h = bb // HBLK
    return h, c * (HBLK * 128) + (bb % HBLK) * 128 + local % 128


def _pad_coords_u(col):
    # global user id -> row in the 8x6272-padded user table
    return (col // SSHARD) * SPAD + col % SSHARD


def _layout(seg_lens, nblocks, nranges):
    """Edge ordering: superblock -> range -> block. Returns (off[b][r], sb
    descriptors, total). All lengths are multiples of 128 and >= 128 so
    every (block, range) has at least one tile (batched PSUM evacuation
    assumes every block's accumulation chain exists)."""
    off = np.zeros((nblocks, nranges), np.int64)
    sbs = []
    pos = 0
    for sb0 in range(0, nblocks, SBSZ):
        blocks = list(range(sb0, min(sb0 + SBSZ, nblocks)))
        sb_start = pos
        rng_info = []
        for r in range(nranges):
            r_start = pos
            for b in blocks:
                off[b, r] = pos
                pos += int(seg_lens[b, r])
            rng_info.append((r_start, pos - r_start))
        sbs.append({"blocks": blocks, "start": sb_start, "end": pos,
                    "ranges": rng_info})
    return off, sbs, pos


def _group_and_fill(block, rng_id, local, within, val, nranges, off, total,
                    sbs, passes):
    """Place edges into the padded global layout. Padding slots keep
    idx=0/dest=0/val=0 (gather row 0, scaled by 0). dest codes are
    128 * (tile index within the pass window, mod WIN) + row-in-block,
    stored fp16 (exact: < 2048)."""
    idx = np.zeros(total, np.int16)
    destp = np.zeros(total, np.float16)
    v = np.zeros(total, np.float32)
    grp = block.astype(np.int64) * nranges + rng_id
    order = np.argsort(grp, kind="stable")
    g_sorted = grp[order]
    uniq, starts = np.unique(g_sorted, return_index=True)
    counts = np.diff(np.append(starts, len(order)))
    base_of = np.repeat(off.reshape(-1)[uniq], counts)
    grp_start_of = np.repeat(starts, counts)
    pos_sorted = base_of + (np.arange(len(order)) - grp_start_of)

    first_of = np.zeros(nranges, np.int64)
    for p in passes:
        for r in p:
            first_of[r] = p[0]
    rstart = np.zeros((len(sbs), nranges), np.int64)
    for i, sb in enumerate(sbs):
        for r in range(nranges):
            rstart[i, r] = sb["ranges"][r][0]
    bo = block[order] // SBSZ
    ro = rng_id[order]
    passstart = rstart[bo, first_of[ro]]
    t = ((pos_sorted - passstart) // 128) % WIN

    idx[pos_sorted] = local[order].astype(np.int16)
    destp[pos_sorted] = (t * 128 + within[order]).astype(np.float16)
    v[pos_sorted] = val[order]
    return idx, destp, v


def _wrap16(a):
    w = np.ascontiguousarray(a.reshape(-1, 16).T)  # [16, E/16]
    return np.tile(w, (8, 1))                      # replicated for 8 Q7 cores


def _wrap128(a):
    return np.ascontiguousarray(a.reshape(-1, 128).T)


def _preprocess(user_emb, item_emb, adj_rows, adj_cols, adj_vals, s_rows,
                s_cols, s_vals):
    ego0 = np.concatenate([np.asarray(user_emb), np.asarray(item_emb)], axis=0)
    ego0_pad = np.zeros((ETAB, EMB), np.float32)  # [2 half-slice tables]
    acc0s = []
    for c in range(NC):
        shard = np.zeros((APAD, EMB), np.float32)
        shard[:ASHARD] = ego0[c * ASHARD:(c + 1) * ASHARD]
        acc0s.append(shard)
        for h in range(2):
            dst = h * HTAB + c * (HBLK * 128)
            ego0_pad[dst:dst + HBLK * 128] = shard[h * HBLK * 128:
                                                   (h + 1) * HBLK * 128]

    adj_rows = np.asarray(adj_rows).astype(np.int64)
    adj_cols = np.asarray(adj_cols).astype(np.int64)
    adj_vals = np.asarray(adj_vals).astype(np.float32)
    s_rows_l = np.asarray(s_rows).astype(np.int64)
    s_cols_l = np.asarray(s_cols).astype(np.int64)
    s_vals_l = np.asarray(s_vals).astype(np.float32)

    # S' = I + S (self edges make u_new = u + S@u a pure segment-sum)
    self_dest = np.arange(U, dtype=np.int64)
    s_rows_l = np.concatenate([s_rows_l, self_dest])
    s_cols_l = np.concatenate([s_cols_l, self_dest])
    s_vals_l = np.concatenate([s_vals_l, np.ones(U, np.float32)])

    per_core = []
    for c in range(NC):
        m = (adj_rows >= c * ASHARD) & (adj_rows < (c + 1) * ASHARD)
        d = adj_rows[m] - c * ASHARD
        col = adj_cols[m]
        # adj sources: users -> u-table coords (ranges 0,1), items -> ego
        # table item half (ranges 2,3)
        is_u = col < U
        up = _pad_coords_u(np.where(is_u, col, 0))
        eh, er = _pad_coords_ego(np.where(is_u, 0, col))
        # item sources sit in rows [25088, 50176) of each half-slice table
        rng_id = np.where(is_u, up // RNG, 2 + eh)
        local = np.where(is_u, up % RNG, er - RNG)
        a = dict(block=d // 128, rng=rng_id, local=local, within=d % 128,
                 val=adj_vals[m])

        m = (s_rows_l >= c * SSHARD) & (s_rows_l < (c + 1) * SSHARD)
        d = s_rows_l[m] - c * SSHARD
        sh, sr = _pad_coords_ego(s_cols_l[m])  # user rows: [0, 25088) per half
        s = dict(block=d // 128, rng=sh, local=sr, within=d % 128,
                 val=s_vals_l[m])
        per_core.append((a, s))

    def seg_max(key, nblocks, nranges):
        lens = np.zeros((nblocks, nranges), np.int64)
        for c in range(NC):
            e = per_core[c][0 if key == "a" else 1]
            cnt = np.bincount(e["block"] * nranges + e["rng"],
                              minlength=nblocks * nranges)
            lens = np.maximum(lens, cnt.reshape(nblocks, nranges))
        return np.maximum(((lens + 127) // 128) * 128, 128)

    a_lens = seg_max("a", ABLOCKS, 4)
    s_lens = seg_max("s", SBLOCKS, 2)
    a_off, a_sbs, a_total = _layout(a_lens, ABLOCKS, 4)
    s_off, s_sbs, s_total = _layout(s_lens, SBLOCKS, 2)

    iota = np.broadcast_to(np.arange(WIN * 128, dtype=np.float16),
                           (128, WIN * 128)).copy()
    in_maps = []
    for c in range(NC):
        a, s = per_core[c]
        aidx, adest, aval = _group_and_fill(a["block"], a["rng"], a["local"],
                                            a["within"], a["val"], 4, a_off,
                                            a_total, a_sbs, A_PASSES)
        sidx, sdest, sval = _group_and_fill(s["block"], s["rng"], s["local"],
                                            s["within"], s["val"], 2, s_off,
                                            s_total, s_sbs, S_PASSES)
        in_maps.append({
            "ego0": ego0_pad,
            "acc0": acc0s[c],
            "aidx": _wrap16(aidx), "adest": _wrap128(adest),
            "aval": _wrap128(aval),
            "sidx": _wrap16(sidx), "sdest": _wrap128(sdest),
            "sval": _wrap128(sval),
            "iota": iota,
        })
    meta = dict(a_lens=a_lens, s_lens=s_lens, a_off=a_off, s_off=s_off,
                a_sbs=a_sbs, s_sbs=s_sbs, a_total=a_total, s_total=s_total)
    return in_maps, meta


def _build(meta):
    import concourse.tile as tile
    from concourse import bacc, mybir

    f32 = mybir.dt.float32
    f16 = mybir.dt.float16
    i16 = mybir.dt.int16
    nc = bacc.Bacc("TRN2", target_bir_lowering=False, debug=False,
                   num_devices=NC, num_swdge_queues=4)

    ego0_d = nc.dram_tensor("ego0", [ETAB, EMB], f32, kind="ExternalInput")
    acc0_d = nc.dram_tensor("acc0", [APAD, EMB], f32, kind="ExternalInput")
    aidx_d = nc.dram_tensor("aidx", [128, meta["a_total"] // 16], i16,
                            kind="ExternalInput")
    adest_d = nc.dram_tensor("adest", [128, meta["a_total"] // 128], f16,
                             kind="ExternalInput")
    aval_d = nc.dram_tensor("aval", [128, meta["a_total"] // 128], f32,
                            kind="ExternalInput")
    sidx_d = nc.dram_tensor("sidx", [128, meta["s_total"] // 16], i16,
                            kind="ExternalInput")
    sdest_d = nc.dram_tensor("sdest", [128, meta["s_total"] // 128], f16,
                             kind="ExternalInput")
    sval_d = nc.dram_tensor("sval", [128, meta["s_total"] // 128], f32,
                            kind="ExternalInput")
    iota_d = nc.dram_tensor("iota", [128, WIN * 128], f16,
                            kind="ExternalInput")
    out_d = nc.dram_tensor("out", [APAD, EMB], f32, kind="ExternalOutput")

    with tile.TileContext(nc) as tc, ExitStack() as ctx:
        persist = ctx.enter_context(tc.tile_pool(name="persist", bufs=1))
        metap = ctx.enter_context(tc.tile_pool(name="meta", bufs=3))
        gpool = ctx.enter_context(tc.tile_pool(name="g", bufs=3))
        gsp = ctx.enter_context(tc.tile_pool(name="gs", bufs=3))
        ohp = ctx.enter_context(tc.tile_pool(name="oh", bufs=5))
        evp = ctx.enter_context(tc.tile_pool(name="ev", bufs=4))
        psp = ctx.enter_context(tc.tile_pool(name="ps", bufs=8, space="PSUM"))
        dram = ctx.enter_context(tc.tile_pool(name="dr", bufs=1, space="DRAM"))

        iota_t = persist.tile([128, WIN, 128], f16, tag="iota")
        nc.sync.dma_start(
            out=iota_t[:],
            in_=iota_d.ap().rearrange("p (w d) -> p w d", d=128))
        acc_t = persist.tile([128, ABLOCKS, EMB], f32, tag="acc")
        nc.sync.dma_start(
            out=acc_t[:],
            in_=acc0_d.ap().rearrange("(b p) e -> p b e", p=128))
        # stashes hold one pass's partial sums until the merge pass; fp16
        # quantization (~5e-4 rel) is far inside the 2e-2 tolerance
        items_buf = persist.tile([128, ABLOCKS, EMB], f16, tag="itemsbuf",
                                 name="items_buf")
        s_stash = persist.tile([128, SBLOCKS, EMB], f16, tag="sstash",
                               name="s_stash")

        # AllGather payloads travel fp16 (halves ncfw link time); a SWDGE
        # cast-DMA expands each gathered table back to the f32 layout the
        # 256B-row dma_gather needs.
        agu_in = [dram.tile([SPAD, EMB], f16, tag=f"agui{l}", name=f"agui{l}")
                  for l in range(NLAYERS)]
        agu_out = [dram.tile([UTAB, EMB], f16, tag=f"aguo{l}", name=f"aguo{l}",
                             addr_space="Shared") for l in range(NLAYERS)]
        agu_tab = [dram.tile([UTAB, EMB], f32, tag=f"agut{l}", name=f"agut{l}")
                   for l in range(NLAYERS)]
        agego_in = [[dram.tile([HBLK * 128, EMB], f16, tag=f"agei{l}h{h}",
                               name=f"agei{l}h{h}") for h in range(2)]
                    for l in range(NLAYERS - 1)]
        agego_out = [[dram.tile([HTAB, EMB], f16, tag=f"ageo{l}h{h}",
                                name=f"ageo{l}h{h}", addr_space="Shared")
                      for h in range(2)] for l in range(NLAYERS - 1)]
        agego_tab = [[dram.tile([HTAB, EMB], f32, tag=f"aget{l}h{h}",
                                name=f"aget{l}h{h}") for h in range(2)]
                     for l in range(NLAYERS - 1)]

        self_q = [0]  # round-robin SWDGE queue counter

        def ag(in_t, out_t, tab_t):
            nc.gpsimd.collective_compute(
                "AllGather", mybir.AluOpType.bypass,
                replica_groups=[list(range(NC))],
                ins=[in_t[:].opt()], outs=[out_t[:].opt()])
            # fp16 -> f32 expansion (SWDGE cast during HBM->HBM DMA)
            nc.gpsimd.dma_start(out=tab_t[:], in_=out_t[:])

        def do_pass(sbs, lens, off, tabs, idx_d, dest_d, val_d, ranges,
                    stash_out=None, stash_in=None, out_fn=None,
                    use_acc=False):
            """One gather + scaled-one-hot + segment-sum pass over a
            contiguous `ranges` subset of the edge layout."""
            for sb in sbs:
                blocks = sb["blocks"]
                nblk = len(blocks)
                b0 = blocks[0]
                e0 = sb["ranges"][ranges[0]][0]
                e1 = sb["ranges"][ranges[-1]][0] + sb["ranges"][ranges[-1]][1]
                ncols = (e1 - e0) // 128
                idx_t = metap.tile([128, (e1 - e0) // 16], i16, tag="idx",
                                   name="idx_t")
                nc.sync.dma_start(out=idx_t[:],
                                  in_=idx_d.ap()[:, e0 // 16:e1 // 16])
                dest_t = metap.tile([128, ncols], f16, tag="dest",
                                    name="dest_t")
                nc.sync.dma_start(out=dest_t[:],
                                  in_=dest_d.ap()[:, e0 // 128:e1 // 128])
                val_t = metap.tile([128, ncols], f32, tag="val", name="val_t")
                nc.sync.dma_start(out=val_t[:],
                                  in_=val_d.ap()[:, e0 // 128:e1 // 128])
                # gathers first on the Pool queue, then the one-hot windows
                # on DVE BEFORE the gather-dependent scale op: the windows
                # only need dest_t, so DVE builds them while SWDGE gathers.
                gts = {}
                for r in ranges:
                    r0, rn = sb["ranges"][r]
                    gt = gpool.tile([128, rn // 128, EMB], f32, tag=f"g{r}",
                                    name=f"g{r}")
                    gts[r] = gt
                    # dma_gather ucode caps at 1024 indices per call; chunk
                    # and spread over the 4 SWDGE queues
                    for c0 in range(0, rn, 1024):
                        n = min(1024, rn - c0)
                        a0, a1 = r0 + c0, r0 + c0 + n
                        nc.gpsimd.dma_gather(
                            out_ap=gt[:, c0 // 128:(c0 + n) // 128, :],
                            in_ap=tabs[r],
                            idxs_ap=idx_t[:, (a0 - e0) // 16:(a1 - e0) // 16],
                            num_idxs=n, num_idxs_reg=n, elem_size=EMB,
                            queue_num=self_q[0] % 4)
                        self_q[0] += 1
                # one-hot windows for the whole (sb, pass) up front; the
                # matmuls then run block-major so each dest block's PSUM
                # accumulation chain is sequential (interleaved start/stop
                # chains within one PSUM bank corrupt each other on HW:
                # `start` clears the bank's has_written tracking).
                ohs = []
                for w0 in range(0, ncols, WIN):
                    wn = min(WIN, ncols - w0)
                    oh = ohp.tile([128, WIN, 128], f16, tag="oh", name="oh")
                    nc.vector.tensor_tensor(
                        oh[:, :wn, :],
                        dest_t[:, w0:w0 + wn].to_broadcast([128, wn, 128]),
                        iota_t[:, :wn, :],
                        mybir.AluOpType.is_equal)
                    ohs.append(oh)
                gs = {}
                for r in ranges:
                    r0, rn = sb["ranges"][r]
                    gs[r] = gsp.tile([128, rn // 128, EMB], f16, tag=f"gs{r}",
                                     name=f"gs{r}")
                    c0 = (r0 - e0) // 128
                    nc.vector.scalar_tensor_tensor(
                        gs[r][:], gts[r][:], 0.0,
                        val_t[:, c0:c0 + rn // 128].to_broadcast(
                            [128, rn // 128, EMB]),
                        mybir.AluOpType.bypass, mybir.AluOpType.mult)
                ps = psp.tile([128, nblk, EMB], f32, name="ps")
                for bi, b in enumerate(blocks):
                    bcols = []
                    for r in ranges:
                        for t in range(int(lens[b][r]) // 128):
                            c = (int(off[b][r]) + t * 128 - e0) // 128
                            gcol = (int(off[b][r]) + t * 128
                                    - sb["ranges"][r][0]) // 128
                            bcols.append((c, r, gcol))
                    for j, (c, r, gcol) in enumerate(bcols):
                        nc.tensor.matmul(
                            ps[:, bi, :],
                            lhsT=ohs[c // WIN][:, c % WIN, :],
                            rhs=gs[r][:, gcol, :],
                            start=(j == 0), stop=(j == len(bcols) - 1),
                            skip_group_check=True)
                if stash_out is not None:
                    # PSUM->stash evacuation on the otherwise-idle ACT
                    # engine keeps the DVE queue free for one-hot windows
                    nc.scalar.copy(stash_out[:, b0:b0 + nblk, :], ps[:])
                    continue
                # ev is fp16: it feeds the fp16 AllGather payloads directly;
                # the acc add reads it mixed-dtype (quantization ~5e-4)
                ev = evp.tile([128, nblk, EMB], f16, tag="ev", name="ev")
                if stash_in is not None:
                    nc.vector.tensor_add(ev[:], ps[:],
                                         stash_in[:, b0:b0 + nblk, :])
                else:
                    nc.vector.tensor_copy(ev[:], ps[:])
                if out_fn is not None:
                    out_fn(b0, nblk, ev)
                if use_acc:
                    nc.vector.tensor_add(acc_t[:, b0:b0 + nblk, :],
                                         acc_t[:, b0:b0 + nblk, :], ev[:])

        for l in range(NLAYERS):
            # the ego "table" is two half-slice tables (blocks 0-48 / 49-97
            # of every shard); users sit in rows [0, RNG), items [RNG, 2RNG)
            if l == 0:
                halves = [ego0_d.ap()[h * HTAB:(h + 1) * HTAB]
                          for h in range(2)]
            else:
                halves = [agego_tab[l - 1][h][:] for h in range(2)]
            tabs_s = [halves[0][0:RNG], halves[1][0:RNG]]
            # S hop: u' = (I+S) @ u over the user rows of both halves.
            # Range 0 reads half 0 (its AllGather was issued at the end of
            # the previous layer); AG(half 1) is issued between the two S
            # passes so its transfer hides behind range-0 compute.
            do_pass(meta["s_sbs"], meta["s_lens"], meta["s_off"], tabs_s,
                    sidx_d, sdest_d, sval_d, ranges=[0], stash_out=s_stash)
            if l > 0:
                ag(agego_in[l - 1][1], agego_out[l - 1][1],
                   agego_tab[l - 1][1])
            def emit_agu(b0, nblk, ev, l=l):
                dst = agu_in[l][:][b0 * 128:(b0 + nblk) * 128, :]
                nc.sync.dma_start(
                    out=dst.rearrange("(b p) e -> p b e", p=128), in_=ev[:])
            do_pass(meta["s_sbs"], meta["s_lens"], meta["s_off"], tabs_s,
                    sidx_d, sdest_d, sval_d, ranges=[1], stash_in=s_stash,
                    out_fn=emit_agu)
            # adjacency hop over [u'; v]; item-source ranges (2,3) don't
            # depend on the u' AllGather, so they run first and the AG
            # transfer hides behind their compute.
            tabs = [agu_tab[l][:][0:RNG], agu_tab[l][:][RNG:2 * RNG],
                    halves[0][RNG:2 * RNG], halves[1][RNG:2 * RNG]]
            do_pass(meta["a_sbs"], meta["a_lens"], meta["a_off"], tabs,
                    aidx_d, adest_d, aval_d, ranges=[2, 3],
                    stash_out=items_buf)
            ag(agu_in[l], agu_out[l], agu_tab[l])
            last = l == NLAYERS - 1
            def emit_ego(b0, nblk, ev, l=l):
                done = 0
                while done < nblk:
                    b = b0 + done
                    h = b // HBLK
                    hb = b % HBLK
                    n = min(nblk - done, HBLK - hb)
                    dst = agego_in[l][h][:][hb * 128:(hb + n) * 128, :]
                    nc.sync.dma_start(
                        out=dst.rearrange("(b p) e -> p b e", p=128),
                        in_=ev[:, done:done + n, :])
                    done += n
            do_pass(meta["a_sbs"], meta["a_lens"], meta["a_off"], tabs,
                    aidx_d, adest_d, aval_d, ranges=[0, 1],
                    stash_in=items_buf, out_fn=None if last else emit_ego,
                    use_acc=True)
            if not last:
                # AG(half 0) now — hides behind the tail of the A-phase
                # compute; AG(half 1) goes out mid-S-phase next layer.
                ag(agego_in[l][0], agego_out[l][0], agego_tab[l][0])

        nc.vector.tensor_scalar_mul(acc_t[:], acc_t[:], 1.0 / (NLAYERS + 1))
        nc.sync.dma_start(
            out=out_d.ap().rearrange("(b p) e -> p b e", p=128), in_=acc_t[:])

    nc.compile()
    return nc


def kernel(user_emb, item_emb, adj_rows, adj_cols, adj_vals, s_rows, s_cols,
           s_vals):
    from concourse.bass_utils import run_bass_kernel_spmd

    in_maps, meta = _preprocess(user_emb, item_emb, adj_rows, adj_cols,
                                adj_vals, s_rows, s_cols, s_vals)
    key = ("v2", meta["a_total"], meta["s_total"])
    if _cache.get("key") != key:
        _cache["nc"] = _build(meta)
        _cache["key"] = key
    res = run_bass_kernel_spmd(_cache["nc"], in_maps,
                               core_ids=list(range(NC)))
    _cache["last_results"] = res
    full = np.empty((NNODES, EMB), np.float32)
    for c in range(NC):
        full[c * ASHARD:(c + 1) * ASHARD] = res.results[c]["out"][:ASHARD]
    return full[:U], full[U:]
